# revision 27
# baseline (speedup 1.0000x reference)
"""Trainium2 Bass kernel for batched CRF negative-log-likelihood (nn_CRF).

Strategy (data-parallel over batch across 8 cores, B_loc=256/core):
  - Exact 4-state reduction of the 6-state CRF (START/STOP rows are -10000 =>
    exp underflows to exactly 0 in f32; first/last steps handled specially).
  - Forward pass in the exp domain: per-step positive 4x4 matrices
    V_t = diag(exp(f_t)) * exp(Tr - kappa + g_t * M); the T-scan is computed
    as chunk-parallel 4x4 matrix-product chains. The diag(exp(f)) factor is
    absorbed as a column scaling of the NEXT step's matrix (one 2x-mode bf16
    multiply per block) so the per-step chain work is a single
    mult(2x)+add(2x)+add(1x) triple on the DVE, with a slice of slots chained
    on the Pool engine in parallel.
  - The 16 exp's of (M[n,p]*g[p] + Trkap[n,p]) run on the Act engine with
    per-call scale/bias immediates; tanh gates come from a host-packed
    pre-activation stream; Act accumulators reduce the gold-path terms.
  - Gold path: host packs onehot(t1) and M[cell]*onehot(t0) into a bf16
    payload and the tag-transition base scores into an f32 stream; the device
    multiplies with on-device f/g and accumulates.
  - Every Ln is deferred to one final batch (a single act-table switch).
"""

import os
import sys
import numpy as np
from contextlib import ExitStack

for _p in ("/opt/trn_rl_repo",):
    if _p not in sys.path:
        sys.path.insert(0, _p)

import concourse.bass as bass
import concourse.tile as tile
from concourse import bacc, mybir
from concourse.bass_utils import run_bass_kernel_spmd

F32 = mybir.dt.float32
BF16 = mybir.dt.bfloat16
I32 = mybir.dt.int32
AF = mybir.ActivationFunctionType
OP = mybir.AluOpType
AX = mybir.AxisListType

K = 4
NT = 6
START, STOP = 4, 5

BF = np.dtype(mybir.dt.np(mybir.dt.bfloat16))


# ---------------- configuration ----------------
class Cfg:
    def __init__(self, B_loc=256, T=2048, NCH=None, TB=None, RB_EVERY=None,
                 SP=None):
        if NCH is None:
            NCH = int(os.environ.get("K_NCH", "64"))
        if TB is None:
            TB = int(os.environ.get("K_TB", "4"))
        if RB_EVERY is None:
            RB_EVERY = int(os.environ.get("K_RB", "8"))
        if SP is None:
            SP = int(os.environ.get("K_SP", "34"))
        self.B_loc = B_loc          # batches per core
        self.T = T
        self.NH = B_loc // 128      # batch "halves" stacked along free dim
        self.NCH = NCH              # chunks per batch (chunk-parallel scan)
        self.L = T // NCH           # steps per chunk
        self.TB = TB                # time-block (steps per streamed block)
        self.NBLK = self.L // TB
        self.RB_EVERY = RB_EVERY    # renormalize Cmat every RB_EVERY blocks
        self.NSL = self.NH * NCH    # slots per partition
        self.SP = SP                # slots chained on the Pool engine
        self.SD = self.NSL - SP     # slots chained on the DVE
        self.NREN = (self.NBLK + RB_EVERY - 1) // RB_EVERY
        assert B_loc % 128 == 0 and T % NCH == 0 and self.L % TB == 0

    def key(self):
        return (self.B_loc, self.T, self.NCH, self.TB, self.RB_EVERY, self.SP)


# ------------- host-side constant prep -------------
def host_consts(transitions, w_shift_in, bias_no, bias_with, w_with_out,
                w_no_out, multiplier):
    Tr = np.asarray(transitions, np.float64)
    mult = np.asarray(multiplier, np.float64)
    # softmax over dim 0 (columns), diagonal then set to -1
    e = np.exp(mult - mult.max(axis=0, keepdims=True))
    Mm = (e / e.sum(axis=0, keepdims=True))
    np.fill_diagonal(Mm, -1.0)

    Tr44 = Tr[:K, :K]
    kappa = float(np.log(np.exp(Tr44).sum(axis=1).mean()))
    consts = np.zeros((128, 56), np.float32)
    # [0:16] chunk-0 first-step V override: exp(Tr[n,START]-kappa) rep over k
    c0v = np.exp(Tr[:K, START] - kappa)
    consts[:, 0:16] = np.repeat(c0v, K)[None, :]
    # [16:20] estop
    consts[:, 16:20] = np.exp(Tr[STOP, :K])[None, :]
    # [20:36] identity (Ct layout [q,k] of I is I)
    consts[:, 20:36] = np.eye(K, dtype=np.float32).reshape(-1)[None, :]
    # [36:52] Trkap row-major (activation bias APs); [52] stays 0.0
    consts[:, 36:52] = (Tr44 - kappa).reshape(-1)[None, :]
    scal = dict(
        kappa=kappa,
        Mm=Mm.astype(np.float64),
        Trkap=(Tr44 - kappa).astype(np.float64),
        TrS=(Tr[:K, START] - kappa).astype(np.float64),
        TrStop=Tr[STOP, :K].astype(np.float64),
        wsh=np.asarray(w_shift_in, np.float64),
        b_no=float(np.asarray(bias_no).reshape(-1)[0]),
        b_with=float(np.asarray(bias_with).reshape(-1)[0]),
        w_w=np.asarray(w_with_out, np.float64),
        w_n=np.asarray(w_no_out, np.float64),
    )
    return consts, scal


# ------------- device program -------------
def build_program(cfg: Cfg, scal, debug=False, rep=1):
    nc = bacc.Bacc("TRN2", target_bir_lowering=False, debug=debug)
    B, T, NH, NCH, L, TB, NBLK = (cfg.B_loc, cfg.T, cfg.NH, cfg.NCH, cfg.L,
                                  cfg.TB, cfg.NBLK)
    NSL, SD, SP, NREN = cfg.NSL, cfg.SD, cfg.SP, cfg.NREN
    CI = NCH * TB            # steps per (h, block)
    HCI = NH * CI            # flattened (h, c, i) block index

    Mm, Trkap = scal["Mm"], scal["Trkap"]

    # inputs host-packed per block: [NBLK, B, NCH, TB, ...]
    f4_d = nc.dram_tensor("f4", [NBLK, B, NCH, TB, K], BF16, kind="ExternalInput")
    uu_d = nc.dram_tensor("uu", [NBLK, B, NCH, TB, K], BF16, kind="ExternalInput")
    ww_d = nc.dram_tensor("ww", [NBLK, B, NCH, TB, K], BF16, kind="ExternalInput")
    pp_d = nc.dram_tensor("pp", [NBLK, B, NCH, TB, 2 * K], BF16, kind="ExternalInput")
    aa_d = nc.dram_tensor("aa", [NBLK, B, NCH, TB], F32, kind="ExternalInput")
    consts_d = nc.dram_tensor("consts", [128, 56], F32, kind="ExternalInput")
    out_d = nc.dram_tensor("nll", [B], F32, kind="ExternalOutput")
    dbg = bool(int(os.environ.get("BASS_DEBUG_DUMP", "0")))
    if dbg:
        dbg_d = {nm: nc.dram_tensor(nm, shp, F32, kind="ExternalOutput")
                 for nm, shp in [("d_gold", [128, NH, NBLK]),
                                 ("d_aacc", [128, NH, NBLK]),
                                 ("d_dotv", [128, NH]), ("d_lsum", [128, NH]),
                                 ("d_mst", [128, (NSL + SP) * NREN]),
                                 ("d_cmat", [128, NSL * 16]),
                                 ("d_tree", [128, NH * 16])]}

    def blk_view(d, j, trail):
        return d.ap()[j].rearrange("(h p) c i" + (" n" if trail else "") +
                                   " -> p h c i" + (" n" if trail else ""), p=128)
    ov = out_d.ap().rearrange("(h p) -> p h", p=128)

    with tile.TileContext(nc) as tc, ExitStack() as ctx:
        ctx.enter_context(nc.allow_low_precision("bf16 chain accumulators"))
        persist = ctx.enter_context(tc.tile_pool(name="persist", bufs=1))
        stream = ctx.enter_context(tc.tile_pool(name="stream", bufs=2))
        work = ctx.enter_context(tc.tile_pool(name="work", bufs=2))
        vpool = ctx.enter_context(tc.tile_pool(name="vpool", bufs=3))
        chain = ctx.enter_context(tc.tile_pool(name="chain", bufs=1))
        fin = ctx.enter_context(tc.tile_pool(name="fin", bufs=1))

        consts = persist.tile([128, 56], F32)
        nc.sync.dma_start(consts[:], consts_d.ap())
        constsb = persist.tile([128, 56], BF16)
        nc.vector.tensor_copy(constsb[:], consts[:])

        for _rep in range(rep):
            CmatD = persist.tile([128, SD, 16], BF16, tag="CmatD")
            CmatP = persist.tile([128, max(SP, 1), 16], BF16, tag="CmatP")
            efc = persist.tile([128, NSL, K], BF16, tag="efc")
            mstoreD = persist.tile([128, SD, NREN], F32, tag="mstoreD")
            mstoreP = persist.tile([128, max(SP, 1), NREN], F32, tag="mstoreP")
            goldacc = persist.tile([128, NH, NBLK], F32, tag="goldacc")
            aacc = persist.tile([128, NH, NBLK], F32, tag="aacc")

            ident = constsb[:, 20:36].unsqueeze(1)
            nc.vector.tensor_copy(CmatD[:], ident.broadcast_to([128, SD, 16]))
            if SP:
                nc.gpsimd.tensor_copy(CmatP[:],
                                      ident.broadcast_to([128, SP, 16]))
                nc.gpsimd.memset(mstoreP[:], 1.0)
            nc.vector.memset(mstoreD[:], 1.0)

            ri = 0

            def prep(j):
                f4 = stream.tile([128, HCI, K], BF16, tag="f4")
                nc.sync.dma_start(f4[:], blk_view(f4_d, j, True))
                uu = stream.tile([128, HCI, K], BF16, tag="uu")
                nc.sync.dma_start(uu[:], blk_view(uu_d, j, True))
                ww = stream.tile([128, HCI, K], BF16, tag="ww")
                nc.sync.dma_start(ww[:], blk_view(ww_d, j, True))
                pp = stream.tile([128, HCI, 2 * K], BF16, tag="pp")
                nc.sync.dma_start(pp[:], blk_view(pp_d, j, True))
                aa = stream.tile([128, HCI], F32, tag="aa")
                nc.sync.dma_start(aa[:], blk_view(aa_d, j, False))

                # gates: g = wsel * tanh(pre)
                g4 = work.tile([128, HCI, K], BF16, tag="g4")
                nc.scalar.activation(g4[:], uu[:], AF.Tanh, bias=consts[:, 52:53])
                nc.vector.tensor_tensor(g4[:], g4[:], ww[:], OP.mult)

                # V1[n,p] = exp(M[n,p]*g[p] + Trkap[n,p]) on Act
                Vt = vpool.tile([128, HCI, 16], BF16, tag="V")
                for n in range(K):
                    for p in range(K):
                        nc.scalar.activation(
                            Vt[:, :, K * n + p], g4[:, :, p], AF.Exp,
                            bias=consts[:, 36 + K * n + p:37 + K * n + p],
                            scale=float(Mm[n, p]))
                ef = vpool.tile([128, HCI, K], BF16, tag="ef")
                nc.scalar.activation(
                    ef[:].rearrange("p x n -> p (x n)"),
                    f4[:].rearrange("p x n -> p (x n)"), AF.Exp,
                    bias=consts[:, 52:53])

                # gold: prod = payload * (f, g); Act accumulates
                prod = work.tile([128, HCI, 2 * K], BF16, tag="prod")
                nc.vector.tensor_tensor(prod[:, :, 0:K], pp[:, :, 0:K],
                                        f4[:], OP.mult)
                nc.vector.tensor_tensor(prod[:, :, K:2 * K], pp[:, :, K:2 * K],
                                        g4[:], OP.mult)
                prodh = prod[:].rearrange("p (h x) l -> p h x l", h=NH)
                aah = aa[:].rearrange("p (h x) -> p h x", h=NH)
                for h in range(NH):
                    nc.scalar.activation(prodh[:, h], prodh[:, h], AF.Copy,
                                         accum_out=goldacc[:, h, j:j + 1])
                    nc.scalar.activation(aah[:, h], aah[:, h], AF.Copy,
                                         accum_out=aacc[:, h, j:j + 1])
                return Vt, ef

            def post(j, Vt, ef):
                # absorb diag(ef) as column scale of next step (issued after
                # the previous block's chain so the DVE never stalls on Act)
                Vr = Vt[:].rearrange("p (s i) v -> p s i v", i=TB)
                Vr4 = Vt[:].rearrange("p (s i) (n k) -> p s i n k", i=TB, k=K)
                efr = ef[:].rearrange("p (s i) k -> p s i k", i=TB)
                for n in range(K):
                    nc.vector.tensor_tensor(
                        Vr4[:, :, 1:, n], Vr4[:, :, 1:, n],
                        efr[:, :, :TB - 1], OP.mult)
                if j > 0:
                    nc.vector.tensor_tensor(
                        Vr4[:, :, 0], Vr4[:, :, 0],
                        efc[:].unsqueeze(2).broadcast_to((128, NSL, K, K)),
                        OP.mult)
                nc.vector.tensor_copy(efc[:], efr[:, :, TB - 1])
                if j == 0:
                    # chunk-0 first step: V = exp(Tr[n,START]-kappa) rep. k
                    c0 = constsb[:, 0:16].unsqueeze(1).broadcast_to([128, NH, 16])
                    nc.vector.tensor_copy(Vr[:, ::NCH, 0], c0)
                return Vr4

            def chain_blk(j, Vr4):
                nonlocal ri
                tmpD = chain.tile([128, SD, K, K, K], BF16, tag="tmpD")
                uD = chain.tile([128, SD, K, K, 2], BF16, tag="uD")
                CDn = CmatD[:].rearrange("p s (q n) -> p s n q", n=K)
                CD4 = CmatD[:].rearrange("p s (q k) -> p s q k", k=K)
                for i in range(TB):
                    for n in range(K):
                        nc.vector.tensor_tensor(
                            tmpD[:, :, n],
                            Vr4[:, 0:SD, i, n].unsqueeze(2)
                               .broadcast_to((128, SD, K, K)),
                            CD4, OP.mult)
                    nc.vector.tensor_tensor(
                        uD[:].rearrange("p s n q k -> p s (n q) k"),
                        tmpD[:, :, :, :, 0:2].rearrange(
                            "p s n q k -> p s (n q) k"),
                        tmpD[:, :, :, :, 2:4].rearrange(
                            "p s n q k -> p s (n q) k"),
                        OP.add)
                    nc.vector.tensor_tensor(
                        CDn, uD[:, :, :, :, 0], uD[:, :, :, :, 1], OP.add)
                if (j + 1) % cfg.RB_EVERY == 0 or j == NBLK - 1:
                    m_t = work.tile([128, SD], F32, tag="m")
                    nc.vector.tensor_copy(m_t[:], CmatD[:, :, 0])
                    r_t = work.tile([128, SD], F32, tag="r")
                    nc.vector.tensor_scalar(
                        r_t[:].bitcast(I32), m_t[:].bitcast(I32),
                        -1, 0x7EF127EA, OP.mult, OP.add)
                    nc.vector.tensor_copy(mstoreD[:, :, ri], r_t[:])
                    rb = work.tile([128, SD], BF16, tag="rb")
                    nc.vector.tensor_copy(rb[:], r_t[:])
                    nc.vector.tensor_tensor(
                        CmatD[:], CmatD[:],
                        rb[:].unsqueeze(2).broadcast_to((128, SD, 16)),
                        OP.mult)
                    ri += 1

            def pool_blk(j, Vr4):
                nonlocal rip
                tmpP = chain.tile([128, SP, K, K, K], BF16, tag="tmpP")
                uP = chain.tile([128, SP, K, K, 2], BF16, tag="uP")
                CPn = CmatP[:].rearrange("p s (q n) -> p s n q", n=K)
                CP4 = CmatP[:].rearrange("p s (q k) -> p s q k", k=K)
                for i in range(TB):
                    for n in range(K):
                        nc.gpsimd.tensor_tensor(
                            tmpP[:, :, n],
                            Vr4[:, SD:, i, n].unsqueeze(2)
                               .broadcast_to((128, SP, K, K)),
                            CP4, OP.mult)
                    nc.gpsimd.tensor_tensor(
                        uP[:].rearrange("p s n q k -> p s (n q) k"),
                        tmpP[:, :, :, :, 0:2].rearrange(
                            "p s n q k -> p s (n q) k"),
                        tmpP[:, :, :, :, 2:4].rearrange(
                            "p s n q k -> p s (n q) k"),
                        OP.add)
                    nc.gpsimd.tensor_tensor(
                        CPn, uP[:, :, :, :, 0], uP[:, :, :, :, 1], OP.add)
                if (j + 1) % cfg.RB_EVERY == 0 or j == NBLK - 1:
                    mp = work.tile([128, SP], F32, tag="mp")
                    nc.gpsimd.tensor_copy(mp[:], CmatP[:, :, 0])
                    rp = work.tile([128, SP], F32, tag="rp")
                    nc.gpsimd.tensor_scalar(
                        rp[:].bitcast(I32), mp[:].bitcast(I32),
                        -1, 0x7EF127EA, OP.mult, OP.add)
                    nc.gpsimd.tensor_copy(mstoreP[:, :, rip], rp[:])
                    rbp = work.tile([128, SP], BF16, tag="rbp")
                    nc.gpsimd.tensor_copy(rbp[:], rp[:])
                    nc.gpsimd.tensor_tensor(
                        CmatP[:], CmatP[:],
                        rbp[:].unsqueeze(2).broadcast_to((128, SP, 16)),
                        OP.mult)
                    rip += 1

            rip = 0
            Vq = []
            for jj in range(NBLK + 2):
                if jj < NBLK:
                    Vq.append(prep(jj))
                if 1 <= jj <= NBLK:
                    Vt_, ef_ = Vq[jj - 1]
                    Vq[jj - 1] = post(jj - 1, Vt_, ef_)
                    if SP:
                        pool_blk(jj - 1, Vq[jj - 1])
                if jj >= 2:
                    chain_blk(jj - 2, Vq[jj - 2])
            assert ri <= NREN and rip <= NREN

            # ---------------- final combine ----------------
            # (scheduled strictly after the block loop: the list scheduler
            # otherwise hoists these into the chain stream where their
            # unsatisfied deps stall the in-order engines)
            ctx_fin = tc.tile_wait_until(1.0 + 2.0 * _rep)
            ctx_fin.__enter__()
            # apply trailing diag(ef_last) row scale
            nc.vector.tensor_tensor(
                CmatD[:].rearrange("p s (q n) -> p s q n", n=K),
                CmatD[:].rearrange("p s (q n) -> p s q n", n=K),
                efc[:, 0:SD].unsqueeze(2).broadcast_to((128, SD, K, K)),
                OP.mult)
            if SP:
                nc.vector.tensor_tensor(
                    CmatP[:].rearrange("p s (q n) -> p s q n", n=K),
                    CmatP[:].rearrange("p s (q n) -> p s q n", n=K),
                    efc[:, SD:].unsqueeze(2).broadcast_to((128, SP, K, K)),
                    OP.mult)

            # tree-combine chunk matrices per half (DVE: h0, Pool: h1)
            treeC0 = fin.tile([128, NCH, 16], BF16, tag="treeC0")
            treeC1 = fin.tile([128, NCH, 16], BF16, tag="treeC1")
            nc.vector.tensor_copy(treeC0[:], CmatD[:, 0:NCH])
            nc.vector.tensor_copy(treeC1[:, 0:SD - NCH], CmatD[:, NCH:SD])
            if SP:
                nc.gpsimd.tensor_copy(treeC1[:, SD - NCH:], CmatP[:])
            NTSC = NCH - 1
            tsc0 = fin.tile([128, NTSC], F32, tag="tsc0")
            tsc1 = fin.tile([128, NTSC], F32, tag="tsc1")
            nc.vector.memset(tsc0[:], 1.0)
            eng1 = nc.gpsimd if SP else nc.vector
            eng1.memset(tsc1[:], 1.0)
            trees = [(treeC0, nc.vector, tsc0), (treeC1, eng1, tsc1)]
            for ti, (treeC, eng, tsc) in enumerate(trees):
                tr4 = treeC[:].rearrange("p c (q k) -> p c q k", k=K)
                ncur = NCH
                lvl = 0
                toff = 0
                while ncur > 1:
                    npair = ncur // 2
                    ttmp = fin.tile([128, npair, K, K, K], BF16,
                                    tag=f"ttmp{ti}_{lvl}")
                    tu = fin.tile([128, npair, K, K, 2], BF16,
                                  tag=f"tu{ti}_{lvl}")
                    # A = C_odd (left), B = C_even (right); A[n,kc]=tr4[c,kc,n]
                    Bev = tr4[:, 0::2][:, 0:npair]
                    for n in range(K):
                        eng.tensor_tensor(
                            ttmp[:, :, n],
                            tr4[:, 1::2][:, 0:npair, :, n].unsqueeze(2)
                                .broadcast_to((128, npair, K, K)),
                            Bev, OP.mult)
                    eng.tensor_tensor(
                        tu[:].rearrange("p c n q k -> p c (n q) k"),
                        ttmp[:, :, :, :, 0:2].rearrange(
                            "p c n q k -> p c (n q) k"),
                        ttmp[:, :, :, :, 2:4].rearrange(
                            "p c n q k -> p c (n q) k"),
                        OP.add)
                    eng.tensor_tensor(
                        tr4[:, 0:npair].rearrange("p c q n -> p c n q"),
                        tu[:, :, :, :, 0], tu[:, :, :, :, 1], OP.add)
                    if npair > 1:
                        # renorm each pair-product by ~1/C[0,0] (bit-trick
                        # reciprocal); exact ln bookkeeping via stored rb
                        tm = fin.tile([128, npair], F32, tag=f"tm{ti}_{lvl}")
                        eng.tensor_copy(tm[:], treeC[:, 0:npair, 0])
                        tr_ = fin.tile([128, npair], F32, tag=f"tr{ti}_{lvl}")
                        eng.tensor_scalar(
                            tr_[:].bitcast(I32), tm[:].bitcast(I32),
                            -1, 0x7EF127EA, OP.mult, OP.add)
                        eng.tensor_copy(tsc[:, toff:toff + npair], tr_[:])
                        trb = fin.tile([128, npair], BF16, tag=f"trb{ti}_{lvl}")
                        eng.tensor_copy(trb[:], tr_[:])
                        eng.tensor_tensor(
                            treeC[:, 0:npair], treeC[:, 0:npair],
                            trb[:].unsqueeze(2).broadcast_to(
                                (128, npair, 16)), OP.mult)
                        toff += npair
                    ncur = npair
                    lvl += 1

            # alpha[h, n] = Ct_total[q=0, n]; dot with estop
            sdot = fin.tile([128, NH, K], BF16, tag="sdot")
            eb = constsb[:, 16:20]
            nc.vector.tensor_tensor(sdot[:, 0], treeC0[:, 0, 0:K], eb, OP.mult)
            nc.vector.tensor_tensor(sdot[:, 1], treeC1[:, 0, 0:K], eb, OP.mult)
            dotv = fin.tile([128, NH], F32, tag="dotv")
            nc.vector.reduce_sum(dotv[:], sdot[:], axis=AX.X)

            # deferred Lns (one act-table switch)
            lnmD = fin.tile([128, SD, NREN], F32, tag="lnmD")
            nc.scalar.activation(lnmD[:].rearrange("p s r -> p (s r)"),
                                 mstoreD[:].rearrange("p s r -> p (s r)"),
                                 AF.Ln, bias=consts[:, 52:53])
            if SP:
                lnmP = fin.tile([128, SP, NREN], F32, tag="lnmP")
                nc.scalar.activation(lnmP[:].rearrange("p s r -> p (s r)"),
                                     mstoreP[:].rearrange("p s r -> p (s r)"),
                                     AF.Ln, bias=consts[:, 52:53])
            fwdp = fin.tile([128, NH], F32, tag="fwdp")
            nc.scalar.activation(fwdp[:], dotv[:], AF.Ln, bias=consts[:, 52:53])

            # lsum[h] = sum of ln(rb) over that half's slots (D and P parts)
            lsum = fin.tile([128, NH], F32, tag="lsum")
            part = fin.tile([128, NH], F32, tag="part")
            nc.vector.memset(part[:], 0.0)
            for h in range(NH):
                lo, hi = h * NCH, (h + 1) * NCH
                dl, dh = lo, min(hi, SD)
                havep = hi > SD
                tgt = part[:, h:h + 1] if havep else lsum[:, h:h + 1]
                nc.vector.reduce_sum(
                    tgt, lnmD[:, dl:dh].rearrange("p s r -> p (s r)"),
                    axis=AX.X)
                if havep:
                    pl, ph = max(lo, SD) - SD, hi - SD
                    nc.vector.reduce_sum(
                        lsum[:, h:h + 1],
                        lnmP[:, pl:ph].rearrange("p s r -> p (s r)"),
                        axis=AX.X)
            nc.vector.tensor_add(lsum[:], lsum[:], part[:])
            lnt = fin.tile([128, NH, NTSC], F32, tag="lnt")
            nc.scalar.activation(lnt[:, 0], tsc0[:], AF.Ln,
                                 bias=consts[:, 52:53])
            nc.scalar.activation(lnt[:, 1], tsc1[:], AF.Ln,
                                 bias=consts[:, 52:53])
            tsum = fin.tile([128, NH], F32, tag="tsum")
            nc.vector.reduce_sum(tsum[:], lnt[:], axis=AX.X)
            nc.vector.tensor_add(lsum[:], lsum[:], tsum[:])
            gtot = fin.tile([128, NH], F32, tag="gtot")
            nc.vector.reduce_sum(gtot[:], goldacc[:], axis=AX.X)
            atot = fin.tile([128, NH], F32, tag="atot")
            nc.vector.reduce_sum(atot[:], aacc[:], axis=AX.X)

            nll = fin.tile([128, NH], F32, tag="nll")
            nc.vector.tensor_sub(nll[:], fwdp[:], lsum[:])
            nc.vector.tensor_sub(nll[:], nll[:], gtot[:])
            nc.vector.tensor_sub(nll[:], nll[:], atot[:])
            nc.sync.dma_start(ov, nll[:])
            if dbg and _rep == 0:
                nc.sync.dma_start(dbg_d["d_gold"].ap(), goldacc[:])
                nc.sync.dma_start(dbg_d["d_aacc"].ap(), aacc[:])
                nc.sync.dma_start(dbg_d["d_dotv"].ap(), dotv[:])
                nc.sync.dma_start(dbg_d["d_lsum"].ap(), lsum[:])
                mstf = fin.tile([128, NSL + SP, NREN], F32, tag="mstf")
                nc.vector.tensor_copy(mstf[:, 0:SD], mstoreD[:])
                nc.vector.tensor_copy(mstf[:, SD:SD + SP], mstoreP[:])
                nc.sync.dma_start(
                    dbg_d["d_mst"].ap().rearrange(
                        "p (s r) -> p s r", r=NREN), mstf[:])
                cmf = fin.tile([128, NSL, 16], F32, tag="cmf")
                nc.vector.tensor_copy(cmf[:, 0:SD], CmatD[:])
                nc.vector.tensor_copy(cmf[:, SD:], CmatP[:])
                nc.sync.dma_start(
                    dbg_d["d_cmat"].ap().rearrange(
                        "p (s v) -> p s v", v=16), cmf[:])
                trf = fin.tile([128, 2, 16], F32, tag="trf")
                nc.vector.tensor_copy(trf[:, 0], treeC0[:, 0])
                nc.vector.tensor_copy(trf[:, 1], treeC1[:, 0])
                nc.sync.dma_start(
                    dbg_d["d_tree"].ap().rearrange("p (h v) -> p h v", v=16),
                    trf[:])
            ctx_fin.__exit__(None, None, None)

    nc.compile()
    return nc


# ------------- host-side stream packing -------------
def host_streams(feats, bias, tags, scal, cfg: Cfg):
    """Build per-core device streams from full [B_loc,T,...] slices."""
    B, T = bias.shape
    NCH, NBLK, TB = cfg.NCH, cfg.NBLK, cfg.TB
    f = np.asarray(feats[:, :, :K], np.float64)
    b = np.asarray(bias, np.float64)
    t1 = np.asarray(tags)
    t0 = np.empty_like(t1)
    t0[:, 1:] = t1[:, :-1]
    t0[:, 0] = 0

    wsh, w_w, w_n = scal["wsh"], scal["w_w"], scal["w_n"]
    b_no, b_with = scal["b_no"], scal["b_with"]
    Mm, Trkap, TrS, TrStop = (scal["Mm"], scal["Trkap"], scal["TrS"],
                              scal["TrStop"])

    withb = b > 0.5
    uu = b[:, :, None] * wsh[None, None, :] + np.where(withb, b_with, b_no)[:, :, None]
    ww = np.where(withb[:, :, None], w_w[None, None, :], w_n[None, None, :])

    oh1 = (t1[:, :, None] == np.arange(K)[None, None, :])
    oh0 = (t0[:, :, None] == np.arange(K)[None, None, :])
    mcell = Mm[t1, t0]
    ohm = oh0 * mcell[:, :, None]
    ohm[:, 0, :] = 0.0
    pp = np.concatenate([oh1.astype(np.float64), ohm], axis=2)

    aa = Trkap[t1, t0]
    aa[:, 0] = TrS[t1[:, 0]]
    aa[:, -1] += TrStop[t1[:, -1]]

    def pack(x, dt):
        trail = x.shape[2:]
        xr = x.reshape(B, NCH, NBLK, TB, *trail)
        order = (2, 0, 1, 3) + tuple(range(4, 4 + len(trail)))
        return np.ascontiguousarray(xr.transpose(*order).astype(dt))

    return dict(f4=pack(f, BF), uu=pack(uu, BF), ww=pack(ww, BF),
                pp=pack(pp, BF), aa=pack(aa, np.float32))


_CACHE = {}


def _get_program(cfg_key, cfg, scal, rep=1):
    key = cfg_key + (rep,)
    if key not in _CACHE:
        _CACHE[key] = build_program(cfg, scal, rep=rep)
    return _CACHE[key]


def _prep(inputs):
    feats = np.ascontiguousarray(np.asarray(inputs["feats"], np.float32))
    bias = np.ascontiguousarray(np.asarray(inputs["bias"], np.float32))
    tags = np.ascontiguousarray(np.asarray(inputs["tags"]).astype(np.int64))
    B, T, _ = feats.shape
    n_cores = 8
    B_loc = B // n_cores
    cfg = Cfg(B_loc=B_loc, T=T)
    consts, scal = host_consts(*[inputs[k] for k in
                                 ("transitions", "w_shift_in", "bias_no",
                                  "bias_with", "w_with_out", "w_no_out",
                                  "multiplier")])
    per_core = []
    for k in range(n_cores):
        sl = slice(k * B_loc, (k + 1) * B_loc)
        st = host_streams(feats[sl], bias[sl], tags[sl], scal, cfg)
        st["consts"] = consts
        per_core.append(st)
    return cfg, scal, consts, per_core


def kernel(feats, bias, tags, transitions, w_shift_in, bias_no, bias_with,
           w_with_out, w_no_out, multiplier):
    inputs = dict(feats=feats, bias=bias, tags=tags, transitions=transitions,
                  w_shift_in=w_shift_in, bias_no=bias_no, bias_with=bias_with,
                  w_with_out=w_with_out, w_no_out=w_no_out,
                  multiplier=multiplier)
    cfg, scal, consts, per_core = _prep(inputs)
    nc = _get_program(cfg.key() + (consts[0].tobytes(),
                                   repr(sorted(scal["wsh"].tolist()))), cfg,
                      scal)
    trace = bool(int(os.environ.get("BASS_KERNEL_TRACE", "0")))
    res = run_bass_kernel_spmd(nc, per_core, core_ids=list(range(8)),
                               trace=trace)
    global LAST_EXEC_NS
    LAST_EXEC_NS = res.exec_time_ns
    out = np.concatenate([r["nll"] for r in res.results], axis=0)
    return out.astype(np.float32)


LAST_EXEC_NS = None


def _time_program(nc, concat_inputs_by_name, iters):
    """Jit one program via shard_map on 8 cores, time with device-resident
    inputs. Returns per-call wall times (ns)."""
    import time
    import jax
    from jax.sharding import Mesh, PartitionSpec, NamedSharding
    from jax.experimental.shard_map import shard_map
    from concourse import bass2jax

    n_cores = 8
    bass2jax.install_neuronx_cc_hook()
    partition_name = nc.partition_id_tensor.name if nc.partition_id_tensor else None
    in_names, out_names, out_avals = [], [], []
    for alloc in nc.m.functions[0].allocations:
        if not isinstance(alloc, mybir.MemoryLocationSet):
            continue
        name = alloc.memorylocations[0].name
        if alloc.kind == "ExternalInput":
            if name != partition_name:
                in_names.append(name)
        elif alloc.kind == "ExternalOutput":
            out_names.append(name)
            out_avals.append(jax.core.ShapedArray(tuple(alloc.tensor_shape),
                                                  mybir.dt.np(alloc.dtype)))
    n_params = len(in_names)
    n_outs = len(out_names)
    in_names_full = list(in_names) + list(out_names)
    if partition_name is not None:
        in_names_full.append(partition_name)

    def _body(*args):
        operands = list(args)
        if partition_name is not None:
            operands.append(bass2jax.partition_id_tensor())
        return tuple(bass2jax._bass_exec_p.bind(
            *operands, out_avals=tuple(out_avals), in_names=tuple(in_names_full),
            out_names=tuple(out_names), lowering_input_output_aliases=(),
            sim_require_finite=True, sim_require_nnan=True, nc=nc))

    devices = jax.devices()[:n_cores]
    mesh = Mesh(np.asarray(devices), ("core",))
    spec = PartitionSpec("core")
    donate = tuple(range(n_params, n_params + n_outs))
    sharded = jax.jit(shard_map(_body, mesh=mesh,
                                in_specs=(spec,) * (n_params + n_outs),
                                out_specs=(spec,) * n_outs,
                                check_rep=False),
                      donate_argnums=donate, keep_unused=True)
    concat_in = [concat_inputs_by_name[nm] for nm in in_names]
    concat_zeros = [np.zeros((n_cores * av.shape[0], *av.shape[1:]), av.dtype)
                    for av in out_avals]
    sh = NamedSharding(mesh, spec)
    dev_in = [jax.device_put(a, sh) for a in concat_in]

    def run_once(timed):
        zs = [jax.device_put(z, sh) for z in concat_zeros]
        jax.block_until_ready(zs)
        t0 = time.perf_counter()
        out = sharded(*dev_in, *zs)
        jax.block_until_ready(out)
        return time.perf_counter() - t0

    run_once(False)
    return np.array([run_once(True) for _ in range(iters)]) * 1e9


def bench(inputs, iters=10):
    """Isolate per-exec device time via rep-scaled programs:
    exec = (t(rep=R) - t(rep=1)) / (R - 1)."""
    cfg, scal, consts, per_core = _prep(inputs)
    names = per_core[0].keys()
    concat = {nm: np.concatenate([pc[nm] for pc in per_core], axis=0)
              for nm in names}
    key = cfg.key() + (consts[0].tobytes(),
                       repr(sorted(scal["wsh"].tolist())))
    R = int(os.environ.get("BENCH_REP", "8"))
    nc1 = _get_program(key, cfg, scal, rep=1)
    t1 = _time_program(nc1, concat, iters)
    print(f"bench rep=1: min={t1.min():.0f} med={np.median(t1):.0f} ns")
    ncR = _get_program(key, cfg, scal, rep=R)
    tR = _time_program(ncR, concat, iters)
    print(f"bench rep={R}: min={tR.min():.0f} med={np.median(tR):.0f} ns")
    exec_ns = (np.median(tR) - np.median(t1)) / (R - 1)
    exec_ns_min = (tR.min() - t1.min()) / (R - 1)
    print(f"per-exec: median-based={exec_ns:.0f}ns min-based={exec_ns_min:.0f}ns")
    return exec_ns


if __name__ == "__main__":
    rng = np.random.default_rng(0)
    B, T = 2048, 2048
    inputs = dict(
        feats=rng.standard_normal((B, T, NT), dtype=np.float32),
        bias=rng.random((B, T), dtype=np.float32),
        tags=rng.integers(0, K, (B, T)).astype(np.int32),
        transitions=rng.standard_normal((NT, NT)).astype(np.float32),
        w_shift_in=rng.standard_normal(K).astype(np.float32),
        bias_no=rng.standard_normal(1).astype(np.float32),
        bias_with=rng.standard_normal(1).astype(np.float32),
        w_with_out=rng.standard_normal(K).astype(np.float32),
        w_no_out=rng.standard_normal(K).astype(np.float32),
        multiplier=rng.standard_normal((K, K)).astype(np.float32),
    )
    out = kernel(**inputs)
    print(out.shape, out[:4])
